# revision 33
# baseline (speedup 1.0000x reference)
"""Trainium2 Bass kernel for edge-softmax attention aggregation (GNN message passing).

Strategy: destination-sharded segment softmax (no cross-core collectives).
  - Host: snake-deal nodes (by degree) into 8 cores x 50 blocks x 4 subblocks
    of 32 node slots, so every subblock receives ~1000 edges; permute edges so
    each subblock owns a contiguous 128*c_sub-slot padded run (partition-major
    interleaved for contiguous per-partition DMA). cutoff/sqrt(dk) is folded
    into q; q|k|v are packed per edge in fp16 (192 cols).
  - Device (per core, SPMD): per 4-subblock batch stream [128, 32, 192] fp16,
    DVE computes per-head logits (fp16 2x-mode mult + pairwise-add tree),
    ACT computes es = exp(w - 2) twice (plain + head-replicated), DVE forms
    es*v and a [128e x 32n] one-hot (is_equal vs iota), and TensorE
    scatter-adds psum[:, strip] += [es*v | es].T @ onehot per 128-edge chunk
    (stationary = edge features, moving = one-hot; fp32 PSUM accumulate).
    Per 128-node block: copy -> PE transpose -> reciprocal -> multiply -> DMA.
  - Host: inverse-permute rows to original node order; zero degree-0 nodes.

Measured: ~289 us HW exec per core (8 cores), rel err ~5e-4 vs fp32 reference.
"""

import sys

if "/opt/trn_rl_repo" not in sys.path:
    sys.path.insert(0, "/opt/trn_rl_repo")

import numpy as np

import concourse.bacc as bacc
import concourse.mybir as mybir
import concourse.tile as tile
from concourse.bass_utils import run_bass_kernel_spmd

F32 = mybir.dt.float32

N_NODES = 50000
N_EDGES = 1_600_000
DK = 64
H = 8
DH = 8  # per-head dim
NC = 8  # cores

SUB_NODES = 32      # node slots per subblock (= one-hot width = matmul M)
SUBS_PER_BLOCK = 4  # PSUM col strips per 128-node block
DEFAULT_BLOCKS = 50  # 128-node blocks per core


F32R = mybir.dt.float32r
BF16 = mybir.dt.bfloat16
FP16 = mybir.dt.float16
GPB = 4  # subblock groups batched per DMA / DVE op (must divide SUBS_PER_BLOCK)


def build_program(c_sub: int, blocks: int, n_cores: int):
    """Build + compile the SPMD Bass program (one program, all cores)."""
    g_core = blocks * SUBS_PER_BLOCK        # subblock groups per core
    slots_sub = 128 * c_sub                 # edge slots per subblock
    cs2 = GPB * c_sub                       # chunks per batch

    nc = bacc.Bacc("TRN2", target_bir_lowering=False, debug=False,
                   num_devices=n_cores)
    qkv = nc.declare_dram_parameter(
        "qkv", [g_core * slots_sub, 3 * DK], FP16, isOutput=False)
    lidx = nc.declare_dram_parameter(
        "lidx", [128, g_core * c_sub], FP16, isOutput=False)
    iota = nc.declare_dram_parameter("iota", [128, SUB_NODES], FP16,
                                     isOutput=False)
    ident = nc.declare_dram_parameter("ident", [DK + 8, DK + 8], F32,
                                      isOutput=False)
    out = nc.declare_dram_parameter("out", [blocks * 128, DK], F32,
                                    isOutput=True)

    with tile.TileContext(nc) as tc:
        with (
            tc.tile_pool(name="const", bufs=1) as cpool,
            tc.tile_pool(name="io", bufs=5) as iopool,
            tc.tile_pool(name="work", bufs=6) as wpool,
            tc.tile_pool(name="nrm", bufs=3) as npool,
            tc.tile_pool(name="psA", bufs=4, space="PSUM") as ppA,
            tc.tile_pool(name="psC", bufs=2, space="PSUM") as ppC,
            tc.tile_pool(name="outp", bufs=4) as opool,
        ):
            iota_t = cpool.tile([128, SUB_NODES], FP16)
            nc.sync.dma_start(iota_t[:], iota[:])
            lidx_t = cpool.tile([128, g_core * c_sub], FP16)
            nc.sync.dma_start(lidx_t[:], lidx[:])
            ident_t = cpool.tile([DK + 8, DK + 8], F32)
            nc.sync.dma_start(ident_t[:], ident[:])
            nbias = cpool.tile([128, 1], F32)
            nc.vector.memset(nbias[:], -2.0)

            blocks_per_batch = GPB // SUBS_PER_BLOCK
            for gb in range(g_core // GPB):   # batch of GPB subblock groups
                psums = [ppA.tile([DK + 8, 128], F32, name=f"psum{i}",
                                  tag=f"psum{i}")
                         for i in range(blocks_per_batch)]

                dt = iopool.tile([128, cs2, 3 * DK], FP16)
                dma_eng = nc.sync if gb % 2 == 0 else nc.scalar
                dma_eng.dma_start(
                    dt[:],
                    qkv[gb * GPB * slots_sub:(gb + 1) * GPB * slots_sub, :]
                    .rearrange("(p s) d -> p s d", p=128),
                )

                # per-edge, per-head logits (fp16 2x-mode tree reduction)
                qk = wpool.tile([128, cs2, DK], FP16)
                nc.vector.tensor_tensor(
                    qk[:], dt[:, :, 0:64], dt[:, :, 64:128],
                    op=mybir.AluOpType.mult)
                qk4 = qk[:].rearrange("p s (h d) -> p s h d", d=DH)
                t1 = wpool.tile([128, cs2, H, 4], FP16)
                nc.vector.tensor_tensor(
                    t1[:], qk4[:, :, :, 0:4], qk4[:, :, :, 4:8],
                    op=mybir.AluOpType.add)
                t2 = wpool.tile([128, cs2, H, 2], FP16)
                nc.vector.tensor_tensor(
                    t2[:], t1[:, :, :, 0:2], t1[:, :, :, 2:4],
                    op=mybir.AluOpType.add)
                w = wpool.tile([128, cs2, H], F32)
                nc.vector.tensor_tensor(
                    w[:].rearrange("p s (h o) -> p s h o", o=1),
                    t2[:, :, :, 0:1], t2[:, :, :, 1:2],
                    op=mybir.AluOpType.add)

                # rhs = [exp(w) | exp(w)*v] : [128, cs2, 72]
                rhs = wpool.tile([128, cs2, DK + 8], FP16)
                nc.scalar.activation(rhs[:, :, 64:72], w[:],
                                     mybir.ActivationFunctionType.Exp,
                                     bias=nbias[:])
                esr = wpool.tile([128, cs2, H, DH], FP16)
                nc.scalar.activation(
                    esr[:],
                    w[:].rearrange("p s (h o) -> p s h o", o=1)
                    .to_broadcast([128, cs2, H, DH]),
                    mybir.ActivationFunctionType.Exp, bias=nbias[:])
                nc.vector.tensor_tensor(
                    rhs[:, :, 0:64].rearrange("p s (h d) -> p s h d", d=DH),
                    dt[:, :, 128:192].rearrange("p s (h d) -> p s h d", d=DH),
                    esr[:],
                    op=mybir.AluOpType.mult)

                # one-hot for all chunks in the batch: one broadcast-compare
                oh = wpool.tile([128, cs2, SUB_NODES], FP16)
                nc.vector.tensor_tensor(
                    oh[:],
                    lidx_t[:, gb * cs2:(gb + 1) * cs2]
                    .rearrange("p (s o) -> p s o", o=1)
                    .to_broadcast([128, cs2, SUB_NODES]),
                    iota_t[:].rearrange("p (o c) -> p o c", o=1)
                    .to_broadcast([128, cs2, SUB_NODES]),
                    op=mybir.AluOpType.is_equal)

                # scatter-add: psum[:, strip_j] += rhs.T @ onehot
                # (stationary = edge features, moving = one-hot)
                for s in range(cs2):
                    psum_t = psums[s // (SUBS_PER_BLOCK * c_sub)]
                    j = (s // c_sub) % SUBS_PER_BLOCK
                    nc.tensor.matmul(
                        psum_t[:, 32 * j:32 * (j + 1)],
                        lhsT=rhs[:, s, :], rhs=oh[:, s, :],
                        start=(s % c_sub == 0), stop=(s % c_sub == c_sub - 1))

                for half in range(blocks_per_batch):
                    b = gb * blocks_per_batch + half
                    psum_t = psums[half]
                    # transpose [72, nodes] -> [nodes, 72], then divide
                    tr_in = npool.tile([DK + 8, 128], F32)
                    nc.scalar.copy(tr_in[:], psum_t[:])
                    psum_o = ppC.tile([128, DK + 8], F32)
                    nc.tensor.transpose(psum_o[:], tr_in[:], ident_t[:])
                    rden = npool.tile([128, H], F32)
                    nc.vector.reciprocal(rden[:], psum_o[:, 64:72])
                    ot = opool.tile([128, H, DH], F32)
                    nc.vector.tensor_tensor(
                        ot[:],
                        psum_o[:, 0:64].rearrange("p (h d) -> p h d", d=DH),
                        rden[:].rearrange("p (h o) -> p h o", o=1)
                        .to_broadcast([128, H, DH]),
                        op=mybir.AluOpType.mult)
                    out_eng = nc.scalar if gb % 2 == 0 else nc.sync
                    out_eng.dma_start(out[b * 128:(b + 1) * 128, :],
                                      ot[:].rearrange("p h d -> p (h d)"))

    nc.compile()
    return nc


def prepare(key, value, query, edge_weight_cutoff, edge_index,
            blocks=DEFAULT_BLOCKS, n_cores=NC):
    """Host-side sharding: node->slot assignment, edge permutation, packing."""
    n_nodes = N_NODES
    n_edges = edge_index.shape[1]
    nsb = n_cores * blocks * SUBS_PER_BLOCK  # total subblocks

    dst = np.asarray(edge_index[1], dtype=np.int64)
    deg = np.bincount(dst, minlength=n_nodes)

    # snake-deal nodes (sorted by degree desc) into nsb bins -> balanced edges
    order_nodes = np.argsort(-deg, kind="stable")
    rounds = -(-n_nodes // nsb)
    assert rounds <= SUB_NODES, "too few subblocks for node count"
    padded = np.full(rounds * nsb, -1, dtype=np.int64)
    padded[:n_nodes] = order_nodes
    arr = padded.reshape(rounds, nsb)
    arr[1::2] = arr[1::2, ::-1]  # snake
    bin_of_node = np.empty(n_nodes, dtype=np.int64)
    slot_of_node = np.empty(n_nodes, dtype=np.int64)
    rr, cc = np.divmod(np.arange(rounds * nsb), nsb)
    flat = arr.reshape(-1)
    mask = flat >= 0
    bin_of_node[flat[mask]] = cc[mask]
    slot_of_node[flat[mask]] = rr[mask]

    bin_edges = np.bincount(bin_of_node[dst], minlength=nsb)
    c_sub = max(1, int(-(-bin_edges.max() // 128)))
    slots_sub = 128 * c_sub

    # group edges by subblock, pad each subblock to slots_sub
    sb_of_edge = bin_of_node[dst]
    eorder = np.argsort(sb_of_edge, kind="stable")
    counts = np.bincount(sb_of_edge, minlength=nsb)
    offsets = np.zeros(nsb + 1, dtype=np.int64)
    np.cumsum(counts, out=offsets[1:])
    sb_sorted = sb_of_edge[eorder]
    rank = np.arange(n_edges, dtype=np.int64) - offsets[sb_sorted]
    # position within the GPB-subblock DMA batch: partition-major interleave
    # so each 128-edge chunk stays subblock-pure under the (p s) device AP
    pp = rank // c_sub
    ss = rank % c_sub
    pos = ((sb_sorted // GPB) * (GPB * slots_sub) + pp * (GPB * c_sub)
           + (sb_sorted % GPB) * c_sub + ss)

    perm = np.full(nsb * slots_sub, n_edges, dtype=np.int64)
    perm[pos] = eorder
    lidx_flat = np.full(nsb * slots_sub, float(SUB_NODES + 7), dtype=np.float16)
    lidx_flat[pos] = slot_of_node[dst[eorder]].astype(np.float16)

    # pack q*cutoff/sqrt(dh) | k and v, all fp16, zero row for padding
    scale = (np.asarray(edge_weight_cutoff, np.float32)
             * np.float32(1.0 / np.sqrt(DH)))
    packed = np.empty((n_edges + 1, 192), dtype=np.float16)
    packed[:n_edges, 0:64] = (np.asarray(query, np.float32)
                              * scale[:, None]).astype(np.float16)
    packed[:n_edges, 64:128] = np.asarray(key, np.float16)
    packed[:n_edges, 128:192] = np.asarray(value, np.float16)
    packed[n_edges] = 0.0

    g_core = blocks * SUBS_PER_BLOCK
    qkv_dev = packed[perm].reshape(n_cores, g_core * slots_sub, 192)
    lidx_dev = (lidx_flat.reshape(n_cores, g_core // GPB, 128, GPB * c_sub)
                .transpose(0, 2, 1, 3).reshape(n_cores, 128, g_core * c_sub))
    lidx_dev = np.ascontiguousarray(lidx_dev)
    iota_np = np.tile(np.arange(SUB_NODES, dtype=np.float16), (128, 1))
    ident_np = np.eye(DK + 8, dtype=np.float32)

    meta = dict(bin_of_node=bin_of_node, slot_of_node=slot_of_node, deg=deg,
                c_sub=c_sub, blocks=blocks, n_cores=n_cores)
    in_maps = [
        {"qkv": qkv_dev[c], "lidx": lidx_dev[c],
         "iota": iota_np, "ident": ident_np}
        for c in range(n_cores)
    ]
    return in_maps, meta


def unshard(results, meta):
    """Gather per-core outputs back to [N_NODES, DK] in original node order."""
    n_cores = meta["n_cores"]
    blocks = meta["blocks"]
    g_core = blocks * SUBS_PER_BLOCK
    allout = np.stack([np.asarray(results[c]["out"]) for c in range(n_cores)])

    bin_of_node = meta["bin_of_node"]
    slot_of_node = meta["slot_of_node"]
    core = bin_of_node // g_core
    g = bin_of_node % g_core
    row = (g // SUBS_PER_BLOCK) * 128 + (g % SUBS_PER_BLOCK) * 32 + slot_of_node
    out_full = allout[core, row].astype(np.float32)
    out_full[meta["deg"] == 0] = 0.0
    return out_full


_program_cache = {}


def kernel(key, value, query, edge_weight_cutoff, edge_index):
    in_maps, meta = prepare(key, value, query, edge_weight_cutoff, edge_index)
    cache_key = (meta["c_sub"], meta["blocks"], meta["n_cores"])
    if cache_key not in _program_cache:
        _program_cache[cache_key] = build_program(*cache_key)
    nc = _program_cache[cache_key]
    res = run_bass_kernel_spmd(nc, in_maps, list(range(meta["n_cores"])))
    return unshard(res.results, meta)


# revision 34
# speedup vs baseline: 1.3882x; 1.3882x over previous
"""Trainium2 Bass kernel for edge-softmax attention aggregation (GNN message passing).

Strategy: destination-sharded segment softmax (no cross-core collectives).
  - Host: snake-deal nodes (by degree) into 8 cores x 50 blocks x 4 subblocks
    of 32 node slots, so every subblock receives ~1000 edges; permute edges so
    each subblock owns a contiguous 128*c_sub-slot padded run (partition-major
    interleaved for contiguous per-partition DMA). cutoff/sqrt(dk) is folded
    into q; q|k|v are packed per edge in fp16 (192 cols).
  - Device (per core, SPMD): per 4-subblock batch stream [128, 32, 192] fp16,
    DVE computes per-head logits (fp16 2x-mode mult + pairwise-add tree),
    ACT computes es = exp(w - 2) twice (plain + head-replicated), DVE forms
    es*v and a [128e x 32n] one-hot (is_equal vs iota), and TensorE
    scatter-adds psum[:, strip] += [es*v | es].T @ onehot per 128-edge chunk
    (stationary = edge features, moving = one-hot; fp32 PSUM accumulate).
    Per 128-node block: copy -> PE transpose -> reciprocal -> multiply -> DMA.
  - Host: inverse-permute rows to original node order; zero degree-0 nodes.

Measured: ~289 us HW exec per core (8 cores), rel err ~5e-4 vs fp32 reference.
"""

import sys

if "/opt/trn_rl_repo" not in sys.path:
    sys.path.insert(0, "/opt/trn_rl_repo")

import numpy as np

import concourse.bacc as bacc
import concourse.mybir as mybir
import concourse.tile as tile
from concourse.bass_utils import run_bass_kernel_spmd

F32 = mybir.dt.float32

N_NODES = 50000
N_EDGES = 1_600_000
DK = 64
H = 8
DH = 8  # per-head dim
NC = 8  # cores

SUB_NODES = 32      # node slots per subblock (= one-hot width = matmul M)
SUBS_PER_BLOCK = 4  # PSUM col strips per 128-node block
DEFAULT_BLOCKS = 50  # 128-node blocks per core


F32R = mybir.dt.float32r
BF16 = mybir.dt.bfloat16
FP16 = mybir.dt.float16
GPB = 4  # subblock groups batched per DMA / DVE op (must divide SUBS_PER_BLOCK)


def build_program(c_sub: int, blocks: int, n_cores: int):
    """Build + compile the SPMD Bass program (one program, all cores)."""
    g_core = blocks * SUBS_PER_BLOCK        # subblock groups per core
    slots_sub = 128 * c_sub                 # edge slots per subblock
    cs2 = GPB * c_sub                       # chunks per batch

    nc = bacc.Bacc("TRN2", target_bir_lowering=False, debug=False,
                   num_devices=n_cores)
    qkv = nc.declare_dram_parameter(
        "qkv", [g_core * slots_sub, 3 * DK], FP16, isOutput=False)
    lidx = nc.declare_dram_parameter(
        "lidx", [128, g_core * c_sub], FP16, isOutput=False)
    iota = nc.declare_dram_parameter("iota", [128, SUB_NODES], FP16,
                                     isOutput=False)
    ident = nc.declare_dram_parameter("ident", [DK + 8, DK + 8], F32,
                                      isOutput=False)
    out = nc.declare_dram_parameter("out", [blocks * 128, DK], F32,
                                    isOutput=True)

    with tile.TileContext(nc) as tc:
        with (
            tc.tile_pool(name="const", bufs=1) as cpool,
            tc.tile_pool(name="io", bufs=5) as iopool,
            tc.tile_pool(name="work", bufs=5) as wpool,
            tc.tile_pool(name="nrm", bufs=3) as npool,
            tc.tile_pool(name="psA", bufs=4, space="PSUM") as ppA,
            tc.tile_pool(name="psC", bufs=2, space="PSUM") as ppC,
            tc.tile_pool(name="outp", bufs=4) as opool,
        ):
            iota_t = cpool.tile([128, SUB_NODES], FP16)
            nc.sync.dma_start(iota_t[:], iota[:])
            lidx_t = cpool.tile([128, g_core * c_sub], FP16)
            nc.sync.dma_start(lidx_t[:], lidx[:])
            ident_t = cpool.tile([DK + 8, DK + 8], F32)
            nc.sync.dma_start(ident_t[:], ident[:])
            nbias = cpool.tile([128, 1], F32)
            nc.vector.memset(nbias[:], -2.0)

            blocks_per_batch = GPB // SUBS_PER_BLOCK
            for gb in range(g_core // GPB):   # batch of GPB subblock groups
                psums = [ppA.tile([DK + 8, 128], F32, name=f"psum{i}",
                                  tag=f"psum{i}")
                         for i in range(blocks_per_batch)]

                dt = iopool.tile([128, cs2, 3 * DK], FP16)
                dma_eng = nc.sync if gb % 2 == 0 else nc.scalar
                dma_eng.dma_start(
                    dt[:],
                    qkv[gb * GPB * slots_sub:(gb + 1) * GPB * slots_sub, :]
                    .rearrange("(p s) d -> p s d", p=128),
                )

                # per-edge, per-head logits (fp16 2x-mode tree reduction)
                qk = wpool.tile([128, cs2, DK], FP16)
                nc.vector.tensor_tensor(
                    qk[:], dt[:, :, 0:64], dt[:, :, 64:128],
                    op=mybir.AluOpType.mult)
                qk4 = qk[:].rearrange("p s (h d) -> p s h d", d=DH)
                t1 = wpool.tile([128, cs2, H, 4], FP16)
                nc.vector.tensor_tensor(
                    t1[:], qk4[:, :, :, 0:4], qk4[:, :, :, 4:8],
                    op=mybir.AluOpType.add)
                t2 = wpool.tile([128, cs2, H, 2], FP16)
                nc.vector.tensor_tensor(
                    t2[:], t1[:, :, :, 0:2], t1[:, :, :, 2:4],
                    op=mybir.AluOpType.add)
                w = wpool.tile([128, cs2, H], F32)
                nc.vector.tensor_tensor(
                    w[:].rearrange("p s (h o) -> p s h o", o=1),
                    t2[:, :, :, 0:1], t2[:, :, :, 1:2],
                    op=mybir.AluOpType.add)

                # rhs = [exp(w) | exp(w)*v] : [128, cs2, 72]
                rhs = wpool.tile([128, cs2, DK + 8], FP16)
                nc.scalar.activation(rhs[:, :, 64:72], w[:],
                                     mybir.ActivationFunctionType.Exp,
                                     bias=nbias[:])
                esr = wpool.tile([128, cs2, H, DH], FP16)
                nc.scalar.activation(
                    esr[:],
                    w[:].rearrange("p s (h o) -> p s h o", o=1)
                    .to_broadcast([128, cs2, H, DH]),
                    mybir.ActivationFunctionType.Exp, bias=nbias[:])
                nc.vector.tensor_tensor(
                    rhs[:, :, 0:64].rearrange("p s (h d) -> p s h d", d=DH),
                    dt[:, :, 128:192].rearrange("p s (h d) -> p s h d", d=DH),
                    esr[:],
                    op=mybir.AluOpType.mult)

                # one-hot for all chunks in the batch: one broadcast-compare
                oh = wpool.tile([128, cs2, SUB_NODES], FP16)
                nc.vector.tensor_tensor(
                    oh[:],
                    lidx_t[:, gb * cs2:(gb + 1) * cs2]
                    .rearrange("p (s o) -> p s o", o=1)
                    .to_broadcast([128, cs2, SUB_NODES]),
                    iota_t[:].rearrange("p (o c) -> p o c", o=1)
                    .to_broadcast([128, cs2, SUB_NODES]),
                    op=mybir.AluOpType.is_equal)

                # scatter-add: psum[:, strip_j] += rhs.T @ onehot
                # (stationary = edge features, moving = one-hot)
                for s in range(cs2):
                    psum_t = psums[s // (SUBS_PER_BLOCK * c_sub)]
                    j = (s // c_sub) % SUBS_PER_BLOCK
                    nc.tensor.matmul(
                        psum_t[:, 32 * j:32 * (j + 1)],
                        lhsT=rhs[:, s, :], rhs=oh[:, s, :],
                        start=(s % c_sub == 0), stop=(s % c_sub == c_sub - 1))

                for half in range(blocks_per_batch):
                    b = gb * blocks_per_batch + half
                    psum_t = psums[half]
                    # transpose [72, nodes] -> [nodes, 72], then divide
                    tr_in = npool.tile([DK + 8, 128], F32)
                    nc.scalar.copy(tr_in[:], psum_t[:])
                    psum_o = ppC.tile([128, DK + 8], F32)
                    nc.tensor.transpose(psum_o[:], tr_in[:], ident_t[:])
                    rden = npool.tile([128, H], F32)
                    nc.vector.reciprocal(rden[:], psum_o[:, 64:72])
                    ot = opool.tile([128, H, DH], F32)
                    nc.vector.tensor_tensor(
                        ot[:],
                        psum_o[:, 0:64].rearrange("p (h d) -> p h d", d=DH),
                        rden[:].rearrange("p (h o) -> p h o", o=1)
                        .to_broadcast([128, H, DH]),
                        op=mybir.AluOpType.mult)
                    nc.sync.dma_start(out[b * 128:(b + 1) * 128, :],
                                      ot[:].rearrange("p h d -> p (h d)"))

    nc.compile()
    return nc


def prepare(key, value, query, edge_weight_cutoff, edge_index,
            blocks=DEFAULT_BLOCKS, n_cores=NC):
    """Host-side sharding: node->slot assignment, edge permutation, packing."""
    n_nodes = N_NODES
    n_edges = edge_index.shape[1]
    nsb = n_cores * blocks * SUBS_PER_BLOCK  # total subblocks

    dst = np.asarray(edge_index[1], dtype=np.int64)
    deg = np.bincount(dst, minlength=n_nodes)

    # snake-deal nodes (sorted by degree desc) into nsb bins -> balanced edges
    order_nodes = np.argsort(-deg, kind="stable")
    rounds = -(-n_nodes // nsb)
    assert rounds <= SUB_NODES, "too few subblocks for node count"
    padded = np.full(rounds * nsb, -1, dtype=np.int64)
    padded[:n_nodes] = order_nodes
    arr = padded.reshape(rounds, nsb)
    arr[1::2] = arr[1::2, ::-1]  # snake
    bin_of_node = np.empty(n_nodes, dtype=np.int64)
    slot_of_node = np.empty(n_nodes, dtype=np.int64)
    rr, cc = np.divmod(np.arange(rounds * nsb), nsb)
    flat = arr.reshape(-1)
    mask = flat >= 0
    bin_of_node[flat[mask]] = cc[mask]
    slot_of_node[flat[mask]] = rr[mask]

    bin_edges = np.bincount(bin_of_node[dst], minlength=nsb)
    c_sub = max(1, int(-(-bin_edges.max() // 128)))
    slots_sub = 128 * c_sub

    # group edges by subblock, pad each subblock to slots_sub
    sb_of_edge = bin_of_node[dst]
    eorder = np.argsort(sb_of_edge, kind="stable")
    counts = np.bincount(sb_of_edge, minlength=nsb)
    offsets = np.zeros(nsb + 1, dtype=np.int64)
    np.cumsum(counts, out=offsets[1:])
    sb_sorted = sb_of_edge[eorder]
    rank = np.arange(n_edges, dtype=np.int64) - offsets[sb_sorted]
    # position within the GPB-subblock DMA batch: partition-major interleave
    # so each 128-edge chunk stays subblock-pure under the (p s) device AP
    pp = rank // c_sub
    ss = rank % c_sub
    pos = ((sb_sorted // GPB) * (GPB * slots_sub) + pp * (GPB * c_sub)
           + (sb_sorted % GPB) * c_sub + ss)

    perm = np.full(nsb * slots_sub, n_edges, dtype=np.int64)
    perm[pos] = eorder
    lidx_flat = np.full(nsb * slots_sub, float(SUB_NODES + 7), dtype=np.float16)
    lidx_flat[pos] = slot_of_node[dst[eorder]].astype(np.float16)

    # pack q*cutoff/sqrt(dh) | k and v, all fp16, zero row for padding
    scale = (np.asarray(edge_weight_cutoff, np.float32)
             * np.float32(1.0 / np.sqrt(DH)))
    packed = np.empty((n_edges + 1, 192), dtype=np.float16)
    packed[:n_edges, 0:64] = (np.asarray(query, np.float32)
                              * scale[:, None]).astype(np.float16)
    packed[:n_edges, 64:128] = np.asarray(key, np.float16)
    packed[:n_edges, 128:192] = np.asarray(value, np.float16)
    packed[n_edges] = 0.0

    g_core = blocks * SUBS_PER_BLOCK
    qkv_dev = packed[perm].reshape(n_cores, g_core * slots_sub, 192)
    lidx_dev = (lidx_flat.reshape(n_cores, g_core // GPB, 128, GPB * c_sub)
                .transpose(0, 2, 1, 3).reshape(n_cores, 128, g_core * c_sub))
    lidx_dev = np.ascontiguousarray(lidx_dev)
    iota_np = np.tile(np.arange(SUB_NODES, dtype=np.float16), (128, 1))
    ident_np = np.eye(DK + 8, dtype=np.float32)

    meta = dict(bin_of_node=bin_of_node, slot_of_node=slot_of_node, deg=deg,
                c_sub=c_sub, blocks=blocks, n_cores=n_cores)
    in_maps = [
        {"qkv": qkv_dev[c], "lidx": lidx_dev[c],
         "iota": iota_np, "ident": ident_np}
        for c in range(n_cores)
    ]
    return in_maps, meta


def unshard(results, meta):
    """Gather per-core outputs back to [N_NODES, DK] in original node order."""
    n_cores = meta["n_cores"]
    blocks = meta["blocks"]
    g_core = blocks * SUBS_PER_BLOCK
    allout = np.stack([np.asarray(results[c]["out"]) for c in range(n_cores)])

    bin_of_node = meta["bin_of_node"]
    slot_of_node = meta["slot_of_node"]
    core = bin_of_node // g_core
    g = bin_of_node % g_core
    row = (g // SUBS_PER_BLOCK) * 128 + (g % SUBS_PER_BLOCK) * 32 + slot_of_node
    out_full = allout[core, row].astype(np.float32)
    out_full[meta["deg"] == 0] = 0.0
    return out_full


_program_cache = {}


def kernel(key, value, query, edge_weight_cutoff, edge_index):
    in_maps, meta = prepare(key, value, query, edge_weight_cutoff, edge_index)
    cache_key = (meta["c_sub"], meta["blocks"], meta["n_cores"])
    if cache_key not in _program_cache:
        _program_cache[cache_key] = build_program(*cache_key)
    nc = _program_cache[cache_key]
    res = run_bass_kernel_spmd(nc, in_maps, list(range(meta["n_cores"])))
    return unshard(res.results, meta)
